# revision 5
# baseline (speedup 1.0000x reference)
"""Trainium2 Bass kernel for nn_DIAGCN (RGCN + GraphConv + classifier over
block-diagonal dialog graphs), SPMD over 8 NeuronCores.

Strategy (v2)
-------------
The dialog graph is a causal 5-tap window (edges i -> i+o, o = 0..4, within
each 100-utterance dialog), and relation_type(i,j) = spk[i]*spk[j] with spk
derived from self-edges.  Every per-node linear map commutes with both the
window sum W(.) (row-mixing) and per-node diagonal scalings, so the whole
network folds into 7-wide channels:

    out = W5(g0) + f0
    g0  = ic0.*W(A0) - ic0s.*W(A0S) + ic1s.*W(A1S) + RA + cA*nvm
    f0  = ic0.*W(B0) - ic0s.*W(B0S) + ic1s.*W(B1S) + FSC + cBc*mask
    A{0,1} = x@(w_rel{0,1}@wA), B likewise with wB; RA = x@(w_root@wA),
    FSC = x@(w_root@wB + w_skip@w_clf); wA = w_gc_rel@w_clf, wB = w_gc_root@w_clf

Per column tile: one [1024 -> 46] bf16 matmul over x (8 k-block matmuls,
k-grouped across tiles so LDWEIGHTS amortizes into the PE reorder window),
a DVE multiply ps[0:32]*spk -> tZ[0:32], a GpSimd copy ps[0:32] -> tZ[32:64]
(plain rows; unused rows are exact zeros via zero columns in Wbig, so the
5-tap window shift-tree runs as ONE [64, W] op per stage with no junk), a
coefficient multiply into tV[0:46], an ACT copy ps[32:46] -> tV[64:78]
(RA|FSC), and ONE [80 -> 39] S-matmul per tile that regroups V rows + RA/FSC
+ host-precomputed nvm/mask const rows into (g0, f0).  ACT/GpSimd copy
(g0, f0) into 4-group packed planes for the final packed 5-tap window (win2),
chunked so only the last ~512 columns run after the last tile.

Layout: nodes sharded by dialog, 64 padded dialogs per core, each dialog =
4 zero gap columns + 100 data columns so window sums never leak.  x ships
transposed+tiled bf16, one ~1 MB DMA per column tile on the sync HWDGE ring
(queued FIRST so the pipeline starts early); weights/constants ride the
scalar ring (wbig first).  Output is the packed [128, 1664] plane, DMA'd in
3 chunks that overlap the tail; the host unpacks.
"""
import numpy as np
import ml_dtypes

BF16 = ml_dtypes.bfloat16

# ---------------------------------------------------------------- constants
B, L, FUT = 500, 100, 4
N = B * L
IN, HID, NCLS = 1024, 512, 7
NCORES = 8
GAP = 4
DLG = L + GAP            # 104 columns per dialog
DPC = 64                 # padded dialogs per core
COLS = DPC * DLG         # 6656 columns per core
NT = 13                  # column tiles
NTC = COLS // NT         # 512
KB = IN // 128           # 8 contraction blocks
M = 46                   # Wbig columns (psum partitions): see layout below
MS = 80                  # S-matmul contraction rows
M2 = 39                  # S-matmul output columns (g0 at 0:7, f0 at 32:39)
GRP = COLS // 4          # 1664 packed-group width (whole dialogs)

# ps rows:  0:7 A0, 7:14 B0, 14:21 A1, 21:28 B1, 28:32 zero, 32:39 RA, 39:46 FSC
# tZ rows:  0:14 [A0S|B0S], 14:28 [A1S|B1S], 28:32 zero, 32:46 [A0|B0] plain,
#           46:60 [A1|B1] plain (unused), 60:64 zero    (all finite!)
# tV rows:  0:46 V = window(tZ)*coef, 46:64 zero (memset once),
#           64:78 [RA|FSC], 78:80 [nvm|mask] consts

D_COUNTS = [63, 63, 63, 63, 62, 62, 62, 62]
D_STARTS = np.concatenate([[0], np.cumsum(D_COUNTS)])[:-1]

# k-groups of tiles sharing each LDWEIGHTS sweep (arrival-aware: small at the
# start while DMA ramps, small at the end so the tail is short)
TGROUPS = [[0], [1], [2, 3], [4, 5, 6], [7, 8, 9], [10, 11], [12]]


def _data_cols():
    d = np.arange(DPC)[:, None]
    u = np.arange(L)[None, :]
    return d * DLG + GAP + u  # [DPC, L]


# ---------------------------------------------------------------- host prep
def _check_graph(edges, relation_type):
    i = np.arange(L)[:, None]
    off = np.arange(FUT + 1)[None, :]
    tl = i + off
    valid = tl < L
    sl = np.broadcast_to(i, tl.shape)[valid]
    tl = tl[valid]
    base = (np.arange(B) * L)[:, None]
    src = (base + sl[None, :]).reshape(-1)
    tgt = (base + tl[None, :]).reshape(-1)
    if edges.shape != (2, src.size) or not (
        np.array_equal(edges[0], src) and np.array_equal(edges[1], tgt)
    ):
        raise ValueError("edge structure does not match the DIAGCN pattern")
    sel = edges[0] == edges[1]
    spk = np.zeros(N, dtype=np.float64)
    spk[edges[0][sel]] = relation_type[sel]
    return spk


def _host_prep(x, edges, relation_type, w_rel, w_root, b_rgcn,
               w_gc_rel, w_gc_root, b_gc, w_skip, b_skip, w_clf, b_clf):
    x = np.asarray(x, dtype=np.float32)
    edges = np.asarray(edges)
    relation_type = np.asarray(relation_type)
    spk = _check_graph(edges, relation_type)

    tgt = edges[1]
    c1 = np.bincount(tgt[relation_type == 1], minlength=N).astype(np.float64)
    c0 = np.bincount(tgt[relation_type == 0], minlength=N).astype(np.float64)
    ic0 = 1.0 / np.maximum(c0, 1.0)
    ic1 = 1.0 / np.maximum(c1, 1.0)
    ic0s = ic0 * spk
    ic1s = ic1 * spk

    f8 = lambda a: np.asarray(a, dtype=np.float64)
    w_rel, w_root, w_gc_rel, w_gc_root, w_skip, w_clf = map(
        f8, (w_rel, w_root, w_gc_rel, w_gc_root, w_skip, w_clf))
    b_rgcn, b_gc, b_skip, b_clf = map(f8, (b_rgcn, b_gc, b_skip, b_clf))

    wA = w_gc_rel @ w_clf
    wB = w_gc_root @ w_clf
    Wbig = np.zeros((IN, M), dtype=np.float64)
    Wbig[:, 0:7] = w_rel[0] @ wA      # A0
    Wbig[:, 7:14] = w_rel[0] @ wB     # B0
    Wbig[:, 14:21] = w_rel[1] @ wA    # A1
    Wbig[:, 21:28] = w_rel[1] @ wB    # B1
    # 28:32 zero
    Wbig[:, 32:39] = w_root @ wA      # RA
    Wbig[:, 39:46] = w_root @ wB + w_skip @ w_clf  # FSC
    # [128 partitions, KB, M]: partition p holds weight rows {k*128+p}
    Wbig = np.ascontiguousarray(
        Wbig.reshape(KB, 128, M).swapaxes(0, 1)).astype(BF16)

    cA = b_rgcn @ wA
    cBc = b_rgcn @ wB + (b_gc + b_skip) @ w_clf + b_clf
    # S-matmul stationary: [MS=80 contraction rows, M2=39 out]
    Sx = np.zeros((MS, M2), dtype=np.float32)
    for i in range(7):
        Sx[0 + i, i] = 1.0           # A0S (coef -ic0s folded in tCF)
        Sx[7 + i, 32 + i] = 1.0      # B0S
        Sx[14 + i, i] = 1.0          # A1S
        Sx[21 + i, 32 + i] = 1.0     # B1S
        Sx[32 + i, i] = 1.0          # A0 plain (coef ic0)
        Sx[39 + i, 32 + i] = 1.0     # B0 plain
        Sx[64 + i, i] = 1.0          # RA
        Sx[71 + i, 32 + i] = 1.0     # FSC
    Sx[78, 0:7] = cA                 # nvm row
    Sx[79, 32:39] = cBc              # mask row
    Sx = Sx.astype(BF16)

    dc = _data_cols()
    mask_col = np.zeros(COLS, dtype=np.float64)
    mask_col[dc.reshape(-1)] = 1.0
    nvm = np.convolve(mask_col, np.ones(FUT + 1))[:COLS] * mask_col
    zc = np.zeros((2, COLS), dtype=np.float32)   # -> tV rows 78:80
    zc[0] = nvm
    zc[1] = mask_col
    zc = zc.astype(BF16)

    in_maps = []
    unshard_info = []
    for c in range(NCORES):
        nd = D_COUNTS[c]
        g0 = D_STARTS[c]
        cols_real = dc[:nd].reshape(-1)
        nodes_real = g0 * L + np.arange(nd * L)

        xt = np.zeros((IN, COLS), dtype=np.float32)
        xt[:, cols_real] = x[nodes_real].T
        # swizzle: [NT][128 partitions][KB][NTC] so each column tile is one
        # DMA with 8 KiB contiguous per partition
        xts = np.ascontiguousarray(
            xt.reshape(KB, 128, NT, NTC).transpose(2, 1, 0, 3)).astype(BF16)

        def vec_to_cols(v):
            out = np.zeros(COLS, dtype=np.float32)
            out[cols_real] = v[nodes_real]
            return out

        spk_c = vec_to_cols(spk)
        ic0_c = vec_to_cols(ic0)
        ic0s_c = vec_to_cols(ic0s)
        ic1s_c = vec_to_cols(ic1s)

        spk32 = np.zeros((32, COLS), dtype=np.float32)
        spk32[0:28] = spk_c          # rows 28:32 multiply exact zeros
        coefrep = np.zeros((M, COLS), dtype=np.float32)
        coefrep[0:7] = -ic0s_c       # A0S
        coefrep[7:14] = -ic0s_c      # B0S
        coefrep[14:21] = ic1s_c      # A1S
        coefrep[21:28] = ic1s_c      # B1S
        coefrep[32:39] = ic0_c       # A0 plain
        coefrep[39:46] = ic0_c       # B0 plain

        in_maps.append(dict(
            xt=xts, wbig=Wbig, sx=Sx, zc=zc,
            spk32=spk32.astype(BF16),
            coefrep=coefrep.astype(BF16),
        ))
        unshard_info.append((nodes_real, cols_real))
    return in_maps, unshard_info


# ---------------------------------------------------------------- bass kernel
_COMPILED = None


def _build():
    import concourse.bass as bass
    from concourse import bacc
    import concourse.mybir as mybir
    from concourse.tile import TileContext

    f32 = mybir.dt.float32
    bf16 = mybir.dt.bfloat16
    ADD = mybir.AluOpType.add
    MUL = mybir.AluOpType.mult

    nc = bacc.Bacc("TRN2", target_bir_lowering=False, debug=False,
                   num_devices=NCORES)
    xt_d = nc.dram_tensor("xt", [NT, 128, KB, NTC], bf16, kind="ExternalInput")
    wbig_d = nc.dram_tensor("wbig", [128, KB, M], bf16, kind="ExternalInput")
    sx_d = nc.dram_tensor("sx", [MS, M2], bf16, kind="ExternalInput")
    zc_d = nc.dram_tensor("zc", [2, COLS], bf16, kind="ExternalInput")
    spk_d = nc.dram_tensor("spk32", [32, COLS], bf16, kind="ExternalInput")
    coef_d = nc.dram_tensor("coefrep", [M, COLS], bf16, kind="ExternalInput")
    y_d = nc.dram_tensor("y", [128, GRP], bf16, kind="ExternalOutput")

    with TileContext(nc) as tc:
        with (
            tc.tile_pool(name="const", bufs=1) as cpool,
            tc.tile_pool(name="xin", bufs=6) as xpool,
            tc.tile_pool(name="wrk", bufs=3) as wpool,
            tc.tile_pool(name="g2", bufs=1) as gpool,
            tc.tile_pool(name="psum", bufs=4, space="PSUM") as ppool,
            tc.tile_pool(name="psum2", bufs=3, space="PSUM") as p2pool,
        ):
            # x tiles stream FIRST on the sync ring so the PE starts ASAP;
            # tile 0 is split into k-halves so matmuls start after 512 KB
            xts = {}
            xts[0] = xpool.tile([128, KB, NTC], bf16, tag="xt", name="xt_t")
            nc.sync.dma_start(xts[0][:, 0:KB // 2, :], xt_d[0, :, 0:KB // 2, :])
            nc.sync.dma_start(xts[0][:, KB // 2:, :], xt_d[0, :, KB // 2:, :])
            xts[1] = xpool.tile([128, KB, NTC], bf16, tag="xt", name="xt_t")
            nc.sync.dma_start(xts[1][:], xt_d[1])

            # constants on the scalar ring, wbig first (gates the first matmul)
            wsb = cpool.tile([128, KB, M], bf16)
            nc.scalar.dma_start(wsb[:], wbig_d[:])
            sxsb = cpool.tile([MS, M2], bf16)
            nc.scalar.dma_start(sxsb[:], sx_d[:])
            tSP = cpool.tile([32, COLS], bf16)   # spk rows 0:28, zeros 28:32
            nc.scalar.dma_start(tSP[:], spk_d[:])
            tCF = cpool.tile([M, COLS], bf16)    # coefficients (rows 28:32 = 0)
            nc.scalar.dma_start(tCF[:], coef_d[:])

            # persistent planes
            tZ = cpool.tile([64, COLS], bf16)    # window input rows
            tT1 = cpool.tile([64, COLS], bf16)   # window stage 1 (persists)
            tV = cpool.tile([MS, COLS], bf16)    # S-matmul rhs plane
            nc.scalar.dma_start(tV[78:80], zc_d[:])  # nvm|mask const rows
            tGP = cpool.tile([128, GRP], bf16)   # packed g0: group g rows 32g..32g+6
            tOP = cpool.tile([128, GRP], bf16)   # packed f0 -> final out
            # tV rows 46:64 must be exact zeros for the S-matmul (rows 32:46
            # are overwritten by the V-multiply later; memset is base-32 legal)
            nc.vector.memset(tV[32:64], 0.0)

            # warm the one-time ACT table load right away using a tiny
            # gpsimd-initialized scratch (no DMA dependency)
            scr = cpool.tile([1, 16], bf16)
            nc.gpsimd.memset(scr[:], 0.0)
            nc.scalar.copy(scr[0:1, 8:16], scr[0:1, 0:8])

            def z_ops(t, ps):
                c0, c1 = t * NTC, (t + 1) * NTC
                # scaled rows (28:32 stay exact zero: ps rows are zero there)
                nc.vector.tensor_tensor(tZ[0:32, c0:c1], ps[0:32], tSP[:, c0:c1], MUL)
                # plain rows + zeros (GPSIMD cannot touch PSUM)
                nc.vector.tensor_copy(tZ[32:64, c0:c1], ps[0:32])
                # RA|FSC pass-through
                nc.scalar.copy(tV[64:78, c0:c1], ps[32:46])

            def window(tlo, thi):
                # 5-tap causal window over the group's columns as a shift tree:
                #   t1 = z + sh1(z); t2 = t1 + sh2(t1); wt = t2 + sh4(z)
                C0, C1 = tlo * NTC, thi * NTC
                W = C1 - C0
                Z = tZ
                T1 = tT1
                T2 = wpool.tile([64, W], bf16, tag="T2")
                WT = wpool.tile([64, W], bf16, tag="WT")
                if tlo == 0:
                    nc.vector.tensor_copy(T1[:, 0:1], Z[:, 0:1])
                    nc.vector.tensor_tensor(T1[:, 1:C1], Z[:, 1:C1], Z[:, 0:C1 - 1], ADD)
                    nc.vector.tensor_copy(T2[:, 0:2], T1[:, 0:2])
                    nc.vector.tensor_tensor(T2[:, 2:], T1[:, 2:C1], T1[:, 0:C1 - 2], ADD)
                    nc.vector.tensor_copy(WT[:, 0:4], T2[:, 0:4])
                    nc.vector.tensor_tensor(WT[:, 4:], T2[:, 4:], Z[:, 0:C1 - 4], ADD)
                else:
                    nc.vector.tensor_tensor(T1[:, C0:C1], Z[:, C0:C1], Z[:, C0 - 1:C1 - 1], ADD)
                    nc.vector.tensor_tensor(T2[:], T1[:, C0:C1], T1[:, C0 - 2:C1 - 2], ADD)
                    nc.vector.tensor_tensor(WT[:], T2[:], Z[:, C0 - 4:C1 - 4], ADD)
                nc.vector.tensor_tensor(tV[0:M, C0:C1], WT[0:M], tCF[:, C0:C1], MUL)

            def finalize(tlo, thi):
                # S-matmul + packed copies for tiles [tlo, thi)
                for t in range(tlo, thi):
                    c0, c1 = t * NTC, (t + 1) * NTC
                    ps2 = p2pool.tile([M2, NTC], f32, tag="ps2", name="ps2")
                    nc.tensor.matmul(ps2[:], sxsb[:], tV[:, c0:c1],
                                     start=True, stop=True)
                    for (glo, ghi) in [(c0, min(c1, (c0 // GRP + 1) * GRP)),
                                       ((c0 // GRP + 1) * GRP, c1)]:
                        if glo >= ghi:
                            continue
                        g = glo // GRP
                        nc.scalar.copy(
                            tGP[32 * g:32 * g + NCLS, glo - g * GRP:ghi - g * GRP],
                            ps2[0:NCLS, glo - c0:ghi - c0])
                        nc.scalar.copy(
                            tOP[32 * g:32 * g + NCLS, glo - g * GRP:ghi - g * GRP],
                            ps2[32:32 + NCLS, glo - c0:ghi - c0])

            # win2: 5-tap window of packed g0, all 4 groups per op (rows
            # 32g..32g+6; other rows are junk and stay junk — host ignores).
            NR = 96 + NCLS
            gs1 = gpool.tile([NR, GRP], bf16, tag="gs1")
            gp = tGP[0:NR]

            def win2_chunk(b0, b1):
                gs2 = gpool.tile([NR, b1 - b0], bf16, tag="gs2")
                gwt = gpool.tile([NR, b1 - b0], bf16, tag="gwt")
                if b0 == 0:
                    nc.vector.tensor_copy(gs1[:, 0:1], gp[:, 0:1])
                    nc.vector.tensor_tensor(gs1[:, 1:b1], gp[:, 1:b1], gp[:, 0:b1 - 1], ADD)
                    nc.vector.tensor_copy(gs2[:, 0:2], gs1[:, 0:2])
                    nc.vector.tensor_tensor(gs2[:, 2:], gs1[:, 2:b1], gs1[:, 0:b1 - 2], ADD)
                    nc.vector.tensor_copy(gwt[:, 0:4], gs2[:, 0:4])
                    nc.vector.tensor_tensor(gwt[:, 4:], gs2[:, 4:], gp[:, 0:b1 - 4], ADD)
                else:
                    nc.vector.tensor_tensor(gs1[:, b0:b1], gp[:, b0:b1], gp[:, b0 - 1:b1 - 1], ADD)
                    nc.vector.tensor_tensor(gs2[:], gs1[:, b0:b1], gs1[:, b0 - 2:b1 - 2], ADD)
                    nc.vector.tensor_tensor(gwt[:], gs2[:], gp[:, b0 - 4:b1 - 4], ADD)
                nc.vector.tensor_tensor(tOP[0:NR, b0:b1], tOP[0:NR, b0:b1], gwt[:], ADD)

            # win2 chunk boundaries: chunk (b0:b1) needs packed cols b0:b1 of
            # ALL 4 groups, i.e. tiles through ceil((3*GRP+b1)/NTC)
            B2 = 12 * NTC - 3 * GRP   # 1152 (ready after tile 11)

            pend_fin = []   # tiles whose finalize is deferred one group
            for gi, grp in enumerate(TGROUPS):
                # k-grouped main matmuls: one LDWEIGHTS per k-block per group
                for t in grp:
                    if t not in xts:
                        xts[t] = xpool.tile([128, KB, NTC], bf16, tag="xt", name="xt_t")
                        nc.sync.dma_start(xts[t][:], xt_d[t])
                pss = {t: ppool.tile([M, NTC], f32, tag="ps", name="ps")
                       for t in grp}
                for k in range(KB):
                    for t in grp:
                        nc.tensor.matmul(
                            pss[t][:], wsb[:, k, :], xts[t][:, k, :],
                            start=(k == 0), stop=(k == KB - 1))
                for t in grp:
                    xts.pop(t)

                # previous group's S-matmuls go ahead of this group's DVE
                # chain in the tensor queue: they're ready, so the PE never
                # idles waiting on the window chain
                if pend_fin:
                    finalize(pend_fin[0], pend_fin[-1] + 1)
                    pend_fin = []

                for t in grp:
                    z_ops(t, pss.pop(t))
                window(grp[0], grp[-1] + 1)
                pend_fin = list(grp)

                # once tiles 0..11 can finalize, run win2 over the bulk and
                # ship it, overlapping tile 12's matmuls and DVE chain
                if grp[-1] == 11:
                    finalize(pend_fin[0], pend_fin[-1] + 1)
                    pend_fin = []
                    win2_chunk(0, B2)
                    nc.sync.dma_start(y_d[:, 0:B2], tOP[:, 0:B2])

            finalize(pend_fin[0], pend_fin[-1] + 1)
            win2_chunk(B2, GRP)
            nc.scalar.dma_start(y_d[:, B2:GRP], tOP[:, B2:GRP])
    nc.compile()
    return nc


def _get_compiled():
    global _COMPILED
    if _COMPILED is None:
        _COMPILED = _build()
    return _COMPILED


def _run(in_maps, trace=False):
    from concourse.bass_utils import run_bass_kernel_spmd
    nc = _get_compiled()
    return run_bass_kernel_spmd(nc, in_maps, list(range(NCORES)), trace=trace)


def kernel(**inputs) -> np.ndarray:
    in_maps, unshard_info = _host_prep(**inputs)
    res = _run(in_maps)
    out = np.zeros((N, NCLS), dtype=np.float32)
    plane = np.empty((NCLS, COLS), dtype=np.float32)
    for c in range(NCORES):
        nodes_real, cols_real = unshard_info[c]
        y = np.asarray(res.results[c]["y"], dtype=np.float32)  # [128, GRP]
        for g in range(4):
            plane[:, g * GRP:(g + 1) * GRP] = y[32 * g:32 * g + NCLS, :]
        out[nodes_real] = plane[:, cols_real].T
    return out
